# revision 41
# baseline (speedup 1.0000x reference)
"""Trainium2 Bass kernel: sliding-window GQA attention block.

Computation (matches the PyTorch/JAX reference):
    q,k,v = x @ {Wq,Wk,Wv}.T ; QK-RMSNorm ; RoPE ; GQA repeat(4x) ;
    softmax(q k^T / sqrt(D) + sliding-window bias(|i-j|<=512)) v ; @ Wo.T

Sharding (no collectives): 8 cores = 2 batches x 4 kv-heads.  Each core
computes K/V for its one kv head and Q/attention for its 4 query heads over
the FULL 2048-token sequence (no K/V halo recompute), then the row-slice of
o_proj for its 512 features -> a full [L, HID] PARTIAL output; the host sums
the 4 partials per batch at gather time (o_proj row-parallel, the hinted
all-reduce happens in the unshard step).

Precision: x and the Q/K/V weights are shipped as a SPLIT-fp8 pair
(e4m3 hi + e5m2 residual, weights prescaled x16 into fp8's normal range -
the scale cancels in RMSNorm for q/k and is folded out of Wo for v), so
each 256-deep hidden-pair contraction is 3 DoubleRow matmuls at 0.5
cycles/row = 25% fewer PE cycles than bf16 at bf16-level accuracy.
Everything else on SBUF is bf16 (host-converted, partition-major [128, ...]
layouts so every DMA is a single long-burst descriptor); PSUM stays fp32.

Attention: Q/K produced in [head_dim, tokens] layout so scores^T and PV
need no transposes; RMSNorm partition reductions are ones-vector matmuls
(1/D folded into the Square activation scale; EPS dropped - mean(q^2)~0.8
>> 1e-6).  Sliding-window edge masks are applied multiplicatively to
exp(scores) with gpsimd affine_select (no bias tables).  Per-(chunk,
key-tile) query ranges are tight, cutting score/PV/exp work ~22% vs
uniform tiles.  The softmax denominator is an exp-sum tree on Pool/DVE
(f32) + one f32r ones-matmul per (chunk, head) instead of a per-tile PE
reduction.

Scheduling: per-engine program order is hand-pipelined.  Cross-engine
normalization chains (rms -> rsqrt -> PE broadcast; denom -> reciprocal ->
PE broadcast -> normalize) are injected into the NEXT unit/head's matmul
stream so the PE never sits behind a DVE/Act round trip.  o_proj token-
tiles are spread one per head-block of the next chunk's attention (convoy
avoidance), with two pieces held back to fill the final chain stall.  A
"bridge" emits the first two attention score+exp tiles inside the
projection phase's PSUM pool across the phase handover, and a dummy exp
pre-loads the Act exp-table while the PE drains Q matmuls.  PSUM: a
start=True matmul wipes its whole 2KB bank, so concurrent accumulation
groups get separate banks; scores(3) + po(2) + pd/pb(1) + o_proj(2) = 8.
"""

import numpy as np


def _ensure_path():
    try:
        import concourse  # noqa: F401
    except ImportError:
        import sys
        for p in ("/opt/trn_rl_repo", "/root/.axon_site/_ro/trn_rl_repo"):
            if p not in sys.path:
                sys.path.insert(0, p)


H, KV, D = 16, 4, 128
GQ = H // KV            # 4 query heads per core (one kv head)
WIN = 512
B, L, HID = 2, 2048, 2048
NHT = HID // 128        # 16 contraction tiles over hidden
NTT = L // 128          # 16 token tiles
CH = 4                  # sequence chunks per core
CHW = L // CH           # 512 tokens per chunk
N_CORES = 8
SWAP_MASK = [p ^ 1 for p in range(32)]


def _plans():
    """Per-chunk list of (kt, q0, qw, mask_lower, mask_upper).

    kt: global 128-key tile; [q0, q0+qw) is the tight chunk-relative query
    range that has any in-window key in the tile.  mask_lower: some q-k>WIN
    pair inside the rectangle (needs select); mask_upper: some k-q>WIN.
    A full-width unmasked tile is sorted first so its PV/denominator matmul
    can initialise every PSUM column with start=True.
    """
    out = []
    for ch in range(CH):
        plans = []
        for kt in range(max(0, 4 * ch - 4), min(NTT, 4 * ch + 8)):
            k0 = kt * 128
            qa0 = max(CHW * ch, k0 - WIN)
            qa1 = min(CHW * ch + CHW, k0 + 128 + WIN)
            if qa1 <= qa0:
                continue
            mlow = (qa1 - 1) - k0 > WIN
            mup = (k0 + 127) - qa0 > WIN
            plans.append((kt, qa0 - CHW * ch, qa1 - qa0, mlow, mup))
        plans.sort(key=lambda t: (t[3] or t[4], -t[2], t[0]))
        assert not plans[0][3] and not plans[0][4] and plans[0][2] == CHW
        out.append(plans)
    return out


PLANS = _plans()

_CACHE = {}


def _build():
    _ensure_path()
    import concourse.mybir as mybir
    import concourse.tile as tile
    from concourse import bacc
    from contextlib import ExitStack

    F32 = mybir.dt.float32
    BF = mybir.dt.bfloat16
    ACTF = mybir.ActivationFunctionType
    ALU = mybir.AluOpType

    nc = bacc.Bacc("TRN2", target_bir_lowering=False, debug=False,
                   num_devices=N_CORES)

    # ---- DRAM I/O (all bf16, partition-major [128, ...]) ----
    F8 = mybir.dt.float8e4
    DR = mybir.MatmulPerfMode.DoubleRow
    E4 = mybir.dt.float8e4
    E5 = mybir.dt.float8e5
    xh = nc.dram_tensor("xh", [128, NHT * L], E4, kind="ExternalInput").ap()
    xl = nc.dram_tensor("xl", [128, NHT * L], E5, kind="ExternalInput").ap()
    wqh = nc.dram_tensor("wqh", [128, NHT * GQ * 128], E4,
                         kind="ExternalInput").ap()
    wql = nc.dram_tensor("wql", [128, NHT * GQ * 128], E5,
                         kind="ExternalInput").ap()
    wkh = nc.dram_tensor("wkh", [128, NHT * 128], E4,
                         kind="ExternalInput").ap()
    wkl = nc.dram_tensor("wkl", [128, NHT * 128], E5,
                         kind="ExternalInput").ap()
    wvh = nc.dram_tensor("wvh", [128, NHT * 128], E4,
                         kind="ExternalInput").ap()
    wvl = nc.dram_tensor("wvl", [128, NHT * 128], E5,
                         kind="ExternalInput").ap()
    woh = nc.dram_tensor("woh", [128, GQ * HID], BF,
                         kind="ExternalInput").ap()
    ckh = nc.dram_tensor("ckh", [128, L], BF, kind="ExternalInput").ap()
    skh = nc.dram_tensor("skh", [128, L], BF, kind="ExternalInput").ap()
    cqh = nc.dram_tensor("cqh", [128, L], BF, kind="ExternalInput").ap()
    sqh = nc.dram_tensor("sqh", [128, L], BF, kind="ExternalInput").ap()
    out = nc.dram_tensor("out", [L, HID], F32, kind="ExternalOutput").ap()

    with tile.TileContext(nc) as tc, ExitStack() as top:
        # ---- persistent SBUF ----
        keep = top.enter_context(tc.tile_pool(name="keep", bufs=1))
        x_sb = keep.tile([128, NHT, L], E4)           # [hid128, k, tok]
        xl_sb = keep.tile([128, NHT, L], E5)
        wq_sb = keep.tile([128, NHT, GQ * 128], E4)   # [hid128, k, feat512]
        wql_sb = keep.tile([128, NHT, GQ * 128], E5)
        wk_sb = keep.tile([128, NHT, 128], E4)
        wkl_sb = keep.tile([128, NHT, 128], E5)
        wv_sb = keep.tile([128, NHT, 128], E4)
        wvl_sb = keep.tile([128, NHT, 128], E5)
        wo_sb = keep.tile([128, GQ, HID], BF)         # [feat128, f, hid]
        ck_sb = keep.tile([128, L], BF)
        sk_sb = keep.tile([128, L], BF)
        cq_sb = keep.tile([128, L], BF)
        sq_sb = keep.tile([128, L], BF)
        kT_sb = keep.tile([128, L], BF)               # [d, tok]
        qT_sb = keep.tile([128, GQ * L], BF)          # [d, h*tok]
        v_sb = keep.tile([128, NTT * 128], BF)        # [tok128, kt*dv]
        aoT_sb = keep.tile([128, GQ, L], BF)          # [dv, h, tok]

        ones32 = keep.tile([128, 128], F32)
        nc.vector.memset(ones32, 1.0)
        onesP = keep.tile([128, 1], BF)               # column of ones
        nc.vector.tensor_copy(onesP, ones32[:, 0:1])
        onesPr = keep.tile([128, 1], mybir.dt.float32r)
        nc.vector.tensor_copy(onesPr, ones32[:, 0:1])
        ones1 = keep.tile([128, 128], BF)             # all-ones (row slices)
        nc.vector.tensor_copy(ones1, ones32)
        warm = keep.tile([128, 16], BF)               # act-table warmup dst

        # ---- input DMAs: x round-robin on 3 queues; weights follow on
        # sync/gpsimd so the Act queue stays clear for the K-norm chain ----
        nc.gpsimd.dma_start(out=wv_sb,
                            in_=wvh.rearrange("p (k f) -> p k f", k=NHT))
        nc.gpsimd.dma_start(out=wvl_sb,
                            in_=wvl.rearrange("p (k f) -> p k f", k=NHT))
        for k in range(NHT):
            eng = (nc.sync, nc.scalar, nc.gpsimd)[k % 3]
            eng.dma_start(out=x_sb[:, k, :], in_=xh[:, k * L:(k + 1) * L])
            eng.dma_start(out=xl_sb[:, k, :], in_=xl[:, k * L:(k + 1) * L])
        nc.gpsimd.dma_start(out=wk_sb,
                            in_=wkh.rearrange("p (k f) -> p k f", k=NHT))
        nc.gpsimd.dma_start(out=wkl_sb,
                            in_=wkl.rearrange("p (k f) -> p k f", k=NHT))
        nc.gpsimd.dma_start(out=ck_sb, in_=ckh)
        nc.gpsimd.dma_start(out=sk_sb, in_=skh)
        nc.sync.dma_start(out=wq_sb,
                          in_=wqh.rearrange("p (k f) -> p k f", k=NHT))
        nc.sync.dma_start(out=wql_sb,
                          in_=wql.rearrange("p (k f) -> p k f", k=NHT))
        nc.sync.dma_start(out=cq_sb, in_=cqh)
        nc.sync.dma_start(out=sq_sb, in_=sqh)
        nc.sync.dma_start(out=wo_sb,
                          in_=woh.rearrange("p (f c) -> p f c", f=GQ))

        es = top.enter_context(tc.tile_pool(name="es", bufs=1))
        sc = top.enter_context(tc.tile_pool(name="sc", bufs=1))
        ys = top.enter_context(tc.tile_pool(name="ys", bufs=1))
        bridge = []

        # ================= V projection =================
        # v[tok, dv] = x^T[tok, hid] @ wv[hid, dv]; a start=True wipes the
        # whole PSUM bank, so concurrent token-tile groups need separate banks
        with ExitStack() as ph:
            pvp = ph.enter_context(tc.tile_pool(name="pvp", bufs=2,
                                                space="PSUM"))
            for i in range(4):
                pvs = [pvp.tile([128, 128], F32, tag=f"pv{t}", name=f"pv{t}")
                       for t in range(4)]
                NP = NHT // 2
                for kp in range(NP):
                    for j in range(4):
                        tt = 4 * i + j
                        xhp = x_sb[:, 2 * kp:2 * kp + 2,
                                   tt * 128:(tt + 1) * 128]
                        xlp = xl_sb[:, 2 * kp:2 * kp + 2,
                                    tt * 128:(tt + 1) * 128]
                        for m, (xp, wp) in enumerate(
                                [(xhp, wv_sb), (xhp, wvl_sb),
                                 (xlp, wv_sb)]):
                            nc.tensor.matmul(
                                pvs[j], xp, wp[:, 2 * kp:2 * kp + 2, :],
                                start=(kp == 0 and m == 0),
                                stop=(kp == NP - 1 and m == 2),
                                perf_mode=DR, skip_group_check=True)
                for j in range(4):
                    tt = 4 * i + j
                    eng = nc.scalar if j % 2 == 0 else nc.vector
                    if j % 2 == 0:
                        nc.scalar.copy(out=v_sb[:, tt * 128:(tt + 1) * 128],
                                       in_=pvs[j])
                    else:
                        nc.vector.tensor_copy(
                            v_sb[:, tt * 128:(tt + 1) * 128], pvs[j])

        # ============ K / Q projection + RMSNorm + RoPE ============
        # units: 4 K chunks then 16 Q (chunk, head) pairs.  The PE parts of
        # each unit's normalization (sum-of-squares reduce, rsqrt broadcast)
        # are injected into the NEXT unit's projection matmul stream so the
        # PE never waits for the Act/DVE chain.
        units = [("k", ch, 0) for ch in range(CH)] + \
                [("q", ch, h) for ch in range(CH) for h in range(GQ)]

        with ExitStack() as ph:
            ppk = ph.enter_context(tc.tile_pool(name="ppk", bufs=2,
                                                space="PSUM"))
            prms = ph.enter_context(tc.tile_pool(name="prms", bufs=3,
                                                 space="PSUM"))
            pprb = ph.enter_context(tc.tile_pool(name="pprb", bufs=2,
                                                 space="PSUM"))
            scr = ph.enter_context(tc.tile_pool(name="scr", bufs=2))

            pend_mid = [None]
            finq = []

            def emit_unit(uidx, kind, ch, h):
                if len(finq) >= 2:
                    finq.pop(0)()
                sl = slice(ch * CHW, (ch + 1) * CHW)
                pk = ppk.tile([128, CHW], F32, tag="p", name="pk")
                NP = NHT // 2
                for kp in range(NP):
                    if kp == 1 and pend_mid[0]:
                        pend_mid[0]()
                        pend_mid[0] = None
                    if kind == "k":
                        wh = wk_sb[:, 2 * kp:2 * kp + 2, :]
                        wl = wkl_sb[:, 2 * kp:2 * kp + 2, :]
                    else:
                        wh = wq_sb[:, 2 * kp:2 * kp + 2,
                                   h * 128:(h + 1) * 128]
                        wl = wql_sb[:, 2 * kp:2 * kp + 2,
                                    h * 128:(h + 1) * 128]
                    xhp = x_sb[:, 2 * kp:2 * kp + 2, ch * CHW:(ch + 1) * CHW]
                    xlp = xl_sb[:, 2 * kp:2 * kp + 2, ch * CHW:(ch + 1) * CHW]
                    for m, (wp, xp) in enumerate(
                            [(wh, xhp), (wl, xhp), (wh, xlp)]):
                        nc.tensor.matmul(pk, wp, xp,
                                         start=(kp == 0 and m == 0),
                                         stop=(kp == NP - 1 and m == 2),
                                         perf_mode=DR)
                # non-PE front half: square (1/D folded into scale), raw copy,
                # rotate-half shuffle, RoPE mul/mul/add
                sq = scr.tile([128, CHW], BF, tag="sq", name="sq")
                nc.scalar.activation(out=sq, in_=pk, func=ACTF.Square,
                                     scale=float(1.0 / np.sqrt(D)))
                raw = scr.tile([128, CHW], BF, tag="raw", name="raw")
                nc.scalar.copy(out=raw, in_=pk)
                swp = scr.tile([128, CHW], BF, tag="swp", name="swp")
                nc.vector.stream_shuffle(out=swp, in_=raw, mask=SWAP_MASK)
                if kind == "k":
                    cT, sT = ck_sb[:, sl], sk_sb[:, sl]
                    dst = kT_sb[:, sl]
                else:
                    cT, sT = cq_sb[:, sl], sq_sb[:, sl]
                    dst = qT_sb[:, h * L + ch * CHW:h * L + (ch + 1) * CHW]
                t1 = scr.tile([128, CHW], BF, tag="t1", name="t1")
                nc.gpsimd.tensor_mul(out=t1, in0=raw, in1=cT)
                t2 = scr.tile([128, CHW], BF, tag="t2", name="t2")
                nc.gpsimd.tensor_mul(out=t2, in0=swp, in1=sT)
                nc.gpsimd.tensor_add(out=t1, in0=t1, in1=t2)

                rms = prms.tile([128, CHW], F32, tag="rms", name="rms")
                ms = scr.tile([128, CHW], F32, tag="ms", name="ms")
                rs = scr.tile([128, CHW], BF, tag="rs", name="rs")

                def mid():
                    # PE partition-reduce of squares, then 1/mean, sqrt
                    nc.tensor.matmul(rms[0:1, :], onesP, sq,
                                     start=True, stop=True,
                                     skip_group_check=True)
                    nc.vector.reciprocal(ms[0:1, :], rms[0:1, :])
                    nc.scalar.activation(out=rs[0:1, :],
                                         in_=ms[0:1, :], func=ACTF.Sqrt)

                def fin():
                    prb = pprb.tile([128, CHW], F32, tag="prb", name="prb")
                    nc.tensor.matmul(prb, ones1[0:1, :], rs[0:1, :],
                                     start=True, stop=True)
                    nc.vector.tensor_mul(out=dst, in0=t1, in1=prb)

                pend_mid[0] = mid
                finq.append(fin)

            for uidx, (kind, ch, h) in enumerate(units):
                emit_unit(uidx, kind, ch, h)
            pend_mid[0]()
            for f in finq:
                f()

        # ================= attention + o_proj =================
        with ExitStack() as ph:
            # scores and o_proj accumulators are temporally disjoint: share
            # one 3-buffer pool so attention gets 3 score banks in flight
            psc = ph.enter_context(tc.tile_pool(name="psc", bufs=3,
                                                space="PSUM"))
            pop = ph.enter_context(tc.tile_pool(name="pop", bufs=2,
                                                space="PSUM"))
            # pd (head h, tile1 of block h+1) and pb (head h, tile4) have
            # disjoint, strictly ordered lifetimes: share one bank
            pdb = ph.enter_context(tc.tile_pool(name="pdb", bufs=1,
                                                space="PSUM"))
            pyp = ph.enter_context(tc.tile_pool(name="pyp", bufs=2,
                                                space="PSUM"))

            fill0 = nc.gpsimd.to_reg(0.0)
            F32R = mybir.dt.float32r

            # deferred normalization chain for head (ch, h): stage1 (exp-sum
            # convert + denominator matmul + reciprocal) and stage2 (PE
            # broadcast + final normalize) are injected into the NEXT head's
            # tile stream so the PE never waits on DVE/Act round trips.
            def make_stage1(esum, ch, h):
                def run():
                    esr = sc.tile([128, CHW], F32R, tag="esr", bufs=2,
                                  name="esr")
                    nc.vector.tensor_copy(esr, esum)
                    pd_t = pdb.tile([128, CHW], F32, tag="pdb", name="pd")
                    nc.tensor.matmul(pd_t[0:1, :], onesPr, esr,
                                     start=True, stop=True,
                                     skip_group_check=True)
                    dr = sc.tile([128, CHW], F32, tag="dr", bufs=2, name="dr")
                    nc.vector.reciprocal(dr[0:1, :], pd_t[0:1, :])
                    drb = sc.tile([128, CHW], BF, tag="drb", bufs=2,
                                  name="drb")
                    nc.gpsimd.tensor_copy(drb[0:1, :], dr[0:1, :])
                    return drb
                return run

            def make_stage2(st1_out, po, ch, h):
                def run():
                    drb = st1_out[0]
                    pb = pdb.tile([128, CHW], F32, tag="pdb", name="pb")
                    nc.tensor.matmul(pb, ones1[0:1, :], drb[0:1, :],
                                     start=True, stop=True)
                    bf = sc.tile([128, CHW], BF, tag="bf", bufs=2, name="bf")
                    nc.scalar.copy(out=bf, in_=pb)
                    sl = slice(ch * CHW, (ch + 1) * CHW)
                    nc.vector.tensor_mul(out=aoT_sb[:, h, sl], in0=po,
                                         in1=bf)
                return run

            pend = [None, None]     # [stage1, stage2] for the previous head

            def head_block(ch, h):
                plans = PLANS[ch]
                po = pop.tile([128, CHW], F32, tag="po", name="po")
                esum = es.tile([128, CHW], F32, tag="esum", bufs=2,
                               name="esum")
                nadd = 0
                use_bridge = ch == 0 and h == 0
                for i, (kt, q0, qw, mlow, mup) in enumerate(plans):
                    if i == 1 and pend[0]:
                        pend[0]()
                        pend[0] = None
                    if i == 4 and pend[1]:
                        pend[1]()
                        pend[1] = None
                    if use_bridge and i < len(bridge):
                        ee = bridge[i]
                        nc.tensor.matmul(po[:, q0:q0 + qw],
                                         v_sb[:, kt * 128:(kt + 1) * 128],
                                         ee[:, :qw],
                                         start=(i == 0),
                                         stop=(i == len(plans) - 1),
                                         skip_group_check=True)
                        if i == 0:
                            nc.vector.tensor_copy(esum, ee)
                        else:
                            nc.gpsimd.tensor_add(out=esum[:, q0:q0 + qw],
                                                 in0=esum[:, q0:q0 + qw],
                                                 in1=ee[:, :qw])
                        continue
                    pscr = psc.tile([128, CHW], F32, tag="ps", name="ps")
                    nc.tensor.matmul(
                        pscr[:, :qw],
                        kT_sb[:, kt * 128:(kt + 1) * 128],
                        qT_sb[:, h * L + ch * CHW + q0:
                              h * L + ch * CHW + q0 + qw],
                        start=True, stop=True)
                    e = es.tile([128, CHW], BF, tag="e", bufs=6, name="e")
                    nc.scalar.activation(out=e[:, :qw], in_=pscr[:, :qw],
                                         func=ACTF.Exp)
                    if mlow or mup:
                        em = es.tile([128, CHW], BF, tag="em", bufs=3,
                                     name="em")
                        if mlow:  # keep where WIN + k - q >= 0
                            base = WIN + kt * 128 - ch * CHW - q0
                            cm, step = 1, -1
                        else:     # keep where WIN - k + q >= 0
                            base = WIN - kt * 128 + ch * CHW + q0
                            cm, step = -1, 1
                        nc.gpsimd.affine_select(
                            out=em[:, :qw], in_=e[:, :qw],
                            pattern=[[step, qw]],
                            compare_op=ALU.is_ge, fill=fill0,
                            base=base, channel_multiplier=cm)
                        ee = em
                    else:
                        ee = e
                    nc.tensor.matmul(po[:, q0:q0 + qw],
                                     v_sb[:, kt * 128:(kt + 1) * 128],
                                     ee[:, :qw],
                                     start=(i == 0),
                                     stop=(i == len(plans) - 1),
                                     skip_group_check=True)
                    # exp-sum tree on Pool/DVE replaces per-tile PE
                    # denominator matmuls
                    if i == 0:
                        nc.vector.tensor_copy(esum, ee)
                    else:
                        eng = nc.gpsimd if nadd % 2 else nc.vector
                        eng.tensor_add(out=esum[:, q0:q0 + qw],
                                       in0=esum[:, q0:q0 + qw],
                                       in1=ee[:, :qw])
                        nadd += 1
                # chain for this head, run inside the next head's stream
                st1_out = [None]
                s1 = make_stage1(esum, ch, h)

                def stage1():
                    st1_out[0] = s1()
                pend[0] = stage1
                pend[1] = make_stage2(st1_out, po, ch, h)

            def emit_oproj(ch, tt, last=False):
                # one token-tile (4 psum groups) of o_proj for chunk ch
                ts = slice(ch * CHW + tt * 128, ch * CHW + (tt + 1) * 128)
                for hc in range(4):
                    py = pyp.tile([128, CHW], F32, tag="py", name="py")
                    hs = slice(hc * 512, (hc + 1) * 512)
                    for f in range(GQ):
                        nc.tensor.matmul(
                            py, aoT_sb[:, f:f + 1, ts][:, 0, :],
                            wo_sb[:, f, hs],
                            start=(f == 0), stop=(f == GQ - 1))
                    y = ys.tile([128, CHW], F32, tag="y", bufs=3, name="y")
                    if last and hc % 2:
                        nc.scalar.copy(out=y, in_=py)
                    else:
                        nc.vector.tensor_copy(y, py)
                    dq = nc.scalar if (last and hc % 2) else nc.sync
                    dq.dma_start(
                        out=out[ch * CHW + tt * 128:ch * CHW + (tt + 1) * 128,
                                hc * 512:(hc + 1) * 512],
                        in_=y)

            # schedule: o_proj(ch) token-tiles are spread one per head block
            # of attention(ch+1), so PE o_proj bursts never starve the Act
            # exp stream (convoy avoidance).  Two pieces of chunk CH-2 are
            # held back to fill the final normalization-chain stall.
            for ch in range(CH):
                for h in range(GQ):
                    head_block(ch, h)
                    if ch > 0 and not (ch == CH - 1 and h >= 2):
                        emit_oproj(ch - 1, h)
            pend[0]()
            emit_oproj(CH - 2, 2, last=True)
            pend[1]()
            emit_oproj(CH - 2, 3, last=True)
            for tt in range(4):
                emit_oproj(CH - 1, tt, last=True)

    nc.compile()
    return nc


def _host_prep(x, cos, sin, Wq, Wk, Wv, Wo, q_norm_w, k_norm_w):
    """Build the 8 per-core input dicts (bf16, partition-major)."""
    import ml_dtypes
    BF16 = ml_dtypes.bfloat16
    scale = 1.0 / np.sqrt(D)
    # interleave head dims [0,64,1,65,...]: rotate_half partners end up on
    # adjacent partitions so the kernel swaps them with one stream-shuffle
    perm = np.empty(D, np.int64)
    perm[0::2] = np.arange(64)
    perm[1::2] = 64 + np.arange(64)

    def rope_tables(w, extra):
        c = (cos * w[None, :] * extra).astype(np.float32)
        w_rot = np.roll(w, -64)
        s = (sin * w_rot[None, :] * extra).astype(np.float32)
        s[:, :64] *= -1.0
        return (np.ascontiguousarray(c.T[perm]).astype(BF16),
                np.ascontiguousarray(s.T[perm]).astype(BF16))

    ckh, skh = rope_tables(k_norm_w, 1.0)
    cqh, sqh = rope_tables(q_norm_w, scale)

    E4 = ml_dtypes.float8_e4m3
    E5 = ml_dtypes.float8_e5m2

    def pmajor(a, dt=BF16):
        # [R, C] with R = rt*128 -> [128, rt*C] (tile-of-128-rows major)
        R, C = a.shape
        return np.ascontiguousarray(
            a.reshape(R // 128, 128, C).transpose(1, 0, 2).reshape(128, -1)
        ).astype(dt)

    WqT, WkT, WvT, WoT = Wq.T, Wk.T, Wv.T, Wo.T
    per_g = []
    for g in range(KV):
        hq = np.arange(GQ) + GQ * g
        idx_q = (hq[:, None] * D + perm[None, :]).ravel()
        idx_k = g * D + perm
        # hi/lo fp8 split: a = e4m3(a) + e5m2(a - e4m3(a)); weights x16
        # into fp8's normal range (cancels in RMSNorm for q/k; folded out
        # of Wo for v)
        wq16 = np.float32(16) * WqT[:, idx_q]
        wk16 = np.float32(16) * WkT[:, idx_k]
        wv16 = np.float32(16) * WvT[:, g * D:(g + 1) * D]
        qh, kh, vh = (a.astype(E4) for a in (wq16, wk16, wv16))
        per_g.append({
            "wqh": pmajor(qh.astype(np.float32), E4),
            "wql": pmajor(wq16 - qh.astype(np.float32), E5),
            "wkh": pmajor(kh.astype(np.float32), E4),
            "wkl": pmajor(wk16 - kh.astype(np.float32), E5),
            "wvh": pmajor(vh.astype(np.float32), E4),
            "wvl": pmajor(wv16 - vh.astype(np.float32), E5),
            "woh": pmajor(WoT[g * GQ * D:(g + 1) * GQ * D, :] / 16.0),
        })
    xb = []
    for b in range(B):
        xT = np.ascontiguousarray(x[b].T)
        xhi = xT.astype(E4)
        xb.append({"xh": pmajor(xhi.astype(np.float32), E4),
                   "xl": pmajor(xT - xhi.astype(np.float32), E5)})

    in_maps = []
    for c in range(N_CORES):
        b, g = divmod(c, KV)
        m = {"ckh": ckh, "skh": skh, "cqh": cqh, "sqh": sqh}
        m.update(xb[b])
        m.update(per_g[g])
        in_maps.append(m)
    return in_maps


def kernel(**inputs):
    _ensure_path()
    from concourse import bass_utils

    if "nc" not in _CACHE:
        _CACHE["nc"] = _build()
    nc = _CACHE["nc"]

    in_maps = _host_prep(
        np.asarray(inputs["x"]), np.asarray(inputs["cos"]),
        np.asarray(inputs["sin"]), np.asarray(inputs["Wq"]),
        np.asarray(inputs["Wk"]), np.asarray(inputs["Wv"]),
        np.asarray(inputs["Wo"]), np.asarray(inputs["q_norm_w"]),
        np.asarray(inputs["k_norm_w"]))

    res = bass_utils.run_bass_kernel_spmd(nc, in_maps,
                                          core_ids=list(range(N_CORES)))
    out = np.zeros((B, L, HID), np.float32)
    for c in range(N_CORES):
        b = c // KV
        out[b] += res.results[c]["out"]
    return out
